# revision 1
# baseline (speedup 1.0000x reference)
"""Trainium2 kernel for nn_CenterDisc (segment_reduce).

Computes: per-class (4 classes) mean of x rows (N=4096 rows of 64x512),
then mean pairwise Frobenius distance between the 4 class centers.

Strategy (data-parallel over N, 8 cores):
  - host: build one-hot(labels) per shard (tiny), shard x rows 512/core
  - device: per-class partial sums via TensorE matmul
        sums[c, d] = sum_k onehot[k, c] * x[k, d]
    streaming 64 MB/core of x from HBM (memory-bound), one-hot is the
    stationary operand (4 cols -> LDWEIGHTS ~free), accumulation over the
    4 row-chunks of 128 in PSUM.
  - host: add the 8 partial (4, 32768) sums, counts = bincount(labels),
    centers + pairwise norms (tiny) on host.
"""

import numpy as np

import concourse.bass as bass
import concourse.tile as tile
from concourse import bacc, mybir
from concourse.bass import ts
from concourse.bass_utils import run_bass_kernel_spmd

# Problem shape (hardcoded per contract)
N, C, PDIM = 4096, 64, 512
D = C * PDIM           # 32768 features per row
NCLS = 4               # num classes
CORES = 8
R = N // CORES         # 512 rows per core
KP = 128               # rows per matmul chunk (partition dim)
KC = R // KP           # 4 k-chunks per core
JB = 2048              # feature columns per block (1 MB DMA per k-chunk)
NB = D // JB           # 16 blocks
MM = 512               # matmul moving free dim (fp32 max / PSUM bank)
JS = JB // MM          # 4 matmul slices per block

_NC_CACHE = None


def _build_bass():
    nc = bacc.Bacc()
    # float32r: same 4-byte layout as fp32 (host arrays stay np.float32),
    # but the PE streams it ~2x faster than fp32's 4 cycles/row.
    mm_dt = mybir.dt.float32r
    x_in = nc.dram_tensor("x", [R, D], mm_dt, kind="ExternalInput")
    oh_in = nc.dram_tensor("onehot", [R, NCLS], mm_dt,
                           kind="ExternalInput")
    out = nc.dram_tensor("sums", [NCLS, D], mybir.dt.float32,
                         kind="ExternalOutput")

    x_r = x_in[:, :].rearrange("(k p) d -> k p d", p=KP)      # (KC, 128, D)
    oh_r = oh_in[:, :].rearrange("(k p) c -> k p c", p=KP)    # (KC, 128, NCLS)

    with tile.TileContext(nc) as tc:
        with (
            tc.tile_pool(name="ohp", bufs=1) as ohp,
            tc.tile_pool(name="xp", bufs=4) as xp,
            tc.tile_pool(name="outp", bufs=2) as outp,
            tc.tile_pool(name="pp", bufs=8, space="PSUM") as pp,
        ):
            ohts = []
            for k in range(KC):
                t = ohp.tile([KP, NCLS], mm_dt, tag=f"oh{k}")
                nc.scalar.dma_start(out=t[:], in_=oh_r[k])
                ohts.append(t)

            for jb in range(NB):
                xts = []
                for k in range(KC):
                    xt = xp.tile([KP, JB], mm_dt, tag=f"x{k}")
                    nc.sync.dma_start(
                        out=xt[:], in_=x_r[k, :, jb * JB:(jb + 1) * JB])
                    xts.append(xt)
                pss = [pp.tile([NCLS, MM], mybir.dt.float32, tag="ps",
                               name=f"ps{jb}_{j}")
                       for j in range(JS)]
                for k in range(KC):
                    for j in range(JS):
                        nc.tensor.matmul(
                            pss[j][:],
                            ohts[k][:],
                            xts[k][:, ts(j, MM)],
                            start=(k == 0),
                            stop=(k == KC - 1),
                        )
                ot = outp.tile([NCLS, JB], mybir.dt.float32, tag="ot")
                for j in range(JS):
                    nc.vector.tensor_copy(out=ot[:, ts(j, MM)], in_=pss[j][:])
                nc.scalar.dma_start(
                    out=out[:, jb * JB:(jb + 1) * JB], in_=ot[:])
    nc.compile()
    return nc


def _get_nc():
    global _NC_CACHE
    if _NC_CACHE is None:
        _NC_CACHE = _build_bass()
    return _NC_CACHE


def _run(x, labels, trace=False, **spmd_kwargs):
    x = np.ascontiguousarray(np.asarray(x, dtype=np.float32).reshape(N, D))
    labels = np.asarray(labels).astype(np.int64)
    onehot = (labels[:, None] == np.arange(NCLS)[None, :]).astype(np.float32)

    in_maps = [
        {"x": x[c * R:(c + 1) * R], "onehot": onehot[c * R:(c + 1) * R]}
        for c in range(CORES)
    ]
    nc = _get_nc()
    last_err = None
    for attempt in range(3):
        try:
            br = run_bass_kernel_spmd(nc, in_maps, core_ids=list(range(CORES)),
                                      trace=trace, **spmd_kwargs)
            break
        except Exception as e:  # transient device wedge (NRT_*) — retry
            last_err = e
            import time as _time
            _time.sleep(3.0)
    else:
        raise last_err

    sums = np.zeros((NCLS, D), dtype=np.float64)
    for r in br.results:
        sums += r["sums"].astype(np.float64)
    counts = np.bincount(labels, minlength=NCLS).astype(np.float64)
    safe = np.maximum(counts, 1.0)
    centers = sums / safe[:, None]                         # (NCLS, D)
    diffs = centers[:, None, :] - centers[None, :, :]      # (NCLS, NCLS, D)
    norms = np.sqrt(np.sum(diffs * diffs, axis=-1))        # (NCLS, NCLS)
    iu, ju = np.triu_indices(NCLS, k=1)
    distance = np.sum(norms[iu, ju]) / len(iu)
    return np.asarray(distance, dtype=np.float32), br


def kernel(x, labels):
    result, _ = _run(x, labels, trace=False)
    return result

